# revision 26
# baseline (speedup 1.0000x reference)
"""Trainium2 Bass kernel for nn_Attention (dense transformer attention block).

Reference computation (shapes fixed):
  x [2, 256, 48, 48] -> RMSNorm over channels -> 1x1 conv to qkv (8 heads, 64 dhead)
  -> prepend 4 learnable mem kv tokens -> softmax attention -> 1x1 conv out [2, 256, 48, 48]

Sharding: 8 cores = 2 batches x 4 head-pairs. Core c handles batch c//4 and
heads (2g, 2g+1), g = c%4. Each core computes its heads' attention and a
partial out-projection [256, 2304] in bf16; partials are ReduceScattered
per 512-column chunk within each batch's 4-core group; each core returns
its 64-channel slice and the host reassembles.

Key structure (v2):
  - RMSNorm sigma folded into the exp: k and v stay RAW; the per-key factor
    sigma_k enters as an exp bias column (ln sigma) and the denominator
    column of the attnv lhsT holds 1/sigma so den = sum exp(S) exactly.
  - exp split across engines: ACT handles 12 of 19 key tiles (table exp,
    bias column), DVE handles 7 via a Schraudolph bf16 bit-trick
    (tensor_scalar fp32->uint16, bitcast to bf16).
  - v produced directly transposed ([pos, dh]) by swapping matmul operands.
  - PE pipeline: sim(jt+1) emitted before attnv(jt) so the PE never queues
    behind an exp; psum: sim pool 2x[128,2,512] + acc pool 4x[128,512].
  - gpsimd queue carries ONLY partition broadcasts + collectives (ordered so
    nothing compute-critical queues behind a blocking collective); all DMAs
    ride sync/scalar; final out DMAs last.
"""
import math

import numpy as np

import concourse.mybir as mybir
import concourse.tile as tile
from concourse import bacc
from concourse.bass_utils import run_bass_kernel_spmd


F32 = mybir.dt.float32
F32R = mybir.dt.float32r
BF16 = mybir.dt.bfloat16
U16 = mybir.dt.uint16
EXP = mybir.ActivationFunctionType.Exp
SQRT = mybir.ActivationFunctionType.Sqrt
LN = mybir.ActivationFunctionType.Ln
SQUARE = mybir.ActivationFunctionType.Square
MULT = mybir.AluOpType.mult
ADD = mybir.AluOpType.add

DIM = 256
HEADS = 8
DHEAD = 64
MEM = 4
HID = 512
N = 48 * 48          # 2304 image positions
NJT = 18             # image key tiles of 128
GROUPS = [[0, 1, 2, 3], [4, 5, 6, 7]]

CHUNKS = [(0, 512), (512, 512), (1024, 512), (1536, 512), (2048, 256)]
NCH = len(CHUNKS)
DVE_JTS = frozenset({2, 4, 7, 9, 11, 14, 16, 17})  # interleaved with ACT

A_SCH = 128.0 / math.log(2.0)        # bf16 Schraudolph slope
B_SCH = 127.0 * 128.0 - 5.5          # minimax-centered magic constant


def _jt_slice(jt):
    """key tile jt -> (chunk index, col offset within chunk)."""
    pos0 = jt * 128
    for ci, (c0, cw) in enumerate(CHUNKS):
        if c0 <= pos0 < c0 + cw:
            return ci, pos0 - c0
    raise AssertionError(jt)


def build():
    nc = bacc.Bacc("TRN2", target_bir_lowering=False, debug=False,
                   enable_asserts=True, num_devices=8)
    x_d = nc.dram_tensor("x", [DIM, N], F32, kind="ExternalInput").ap()
    wqkv_d = nc.dram_tensor("wqkv", [DIM, 384], F32, kind="ExternalInput").ap()
    memk_d = nc.dram_tensor("memk", [128, MEM], F32, kind="ExternalInput").ap()
    memvst_d = nc.dram_tensor("memvst", [MEM, 2, 128], F32,
                              kind="ExternalInput").ap()
    woutT_d = nc.dram_tensor("woutT", [2, DHEAD, DIM], F32,
                             kind="ExternalInput").ap()
    out_d = nc.dram_tensor("out", [DHEAD, N], BF16, kind="ExternalOutput").ap()

    with tile.TileContext(nc) as tc:
        with (
            tc.tile_pool(name="consts", bufs=1) as consts,
            tc.tile_pool(name="big", bufs=1) as big,
            tc.tile_pool(name="io", bufs=2) as io,
            tc.tile_pool(name="wk", bufs=2) as wk,
            tc.tile_pool(name="ps_s", bufs=2, space="PSUM") as ps_s,
            tc.tile_pool(name="ps_a", bufs=4, space="PSUM") as ps_a,
            tc.tile_pool(name="dram", bufs=1, space="DRAM") as dram,
        ):
            # ---------------- input DMAs first (sync/scalar queues) --------
            # per-chunk tiles so chunk-0 compute starts as soon as its DMA
            # lands (whole-tile dependency granularity).
            xs = [[None] * NCH, [None] * NCH]
            dq = [nc.sync, nc.scalar, nc.gpsimd]
            for ci, (c0, cw) in enumerate(CHUNKS):
                for kt in range(2):
                    t = big.tile([128, 512], F32, tag=f"x{kt}_{ci}",
                                 name=f"x{kt}_{ci}")
                    xs[kt][ci] = t
                    dq[(2 * ci + kt) % 3].dma_start(
                        out=t[:, 0:cw],
                        in_=x_d[128 * kt:128 * kt + 128, c0:c0 + cw])
            wq_f = io.tile([128, 2, 384], F32, tag="wq_f")
            nc.sync.dma_start(out=wq_f[:, 0, :], in_=wqkv_d[0:128, :])
            nc.sync.dma_start(out=wq_f[:, 1, :], in_=wqkv_d[128:256, :])
            memk_f = io.tile([128, MEM], F32, tag="memk_f")
            nc.sync.dma_start(out=memk_f[:, :], in_=memk_d)
            memv_f = io.tile([MEM, 2, 128], F32, tag="memv_f")
            nc.sync.dma_start(out=memv_f[:, :, :], in_=memvst_d)
            woutA_f = io.tile([128, DIM], F32, tag="woutA_f")
            woutB_f = io.tile([128, DIM], F32, tag="woutB_f")
            nc.scalar.dma_start(out=woutA_f[64:128, :], in_=woutT_d[0, :, :])
            nc.scalar.dma_start(out=woutB_f[64:128, :], in_=woutT_d[1, :, :])

            # ---------------- constants / staging init --------------------
            ones_f = consts.tile([128, 1], F32)
            nc.vector.memset(ones_f[:, :], 1.0)
            ones_r = consts.tile([128, 128], F32R)
            nc.vector.tensor_copy(ones_r[:, :],
                                  ones_f[:, :].to_broadcast((128, 128)))

            # staging tiles for attnv lhsT: [keys, 2 heads, 128 cols]
            # cols: [0] = 1/sigma (denominator), [1:64] zeros, [64:128] = v^T
            vstag = [big.tile([128, 2, 128], BF16, tag=f"vst{jt}", name=f"vst{jt}")
                     for jt in range(NJT + 1)]
            for jt in range(NJT + 1):
                nc.gpsimd.memset(vstag[jt][:, :, :], 0.0)

            # per-key column tensors use duplicated columns: index 2*jt
            siginv = consts.tile([128, 2 * NJT], F32, tag="siginv")
            lnss = consts.tile([128, 2 * NJT], F32, tag="lnss")
            lnsig = consts.tile([128, 2 * NJT + 1], F32, tag="lnsig")
            nc.vector.memset(lnsig[:, 2 * NJT:2 * NJT + 1], 0.0)  # mem bias
            dve_bias = consts.tile([128, 2 * NJT], F32, tag="dve_bias")

            # ---------------- collective warmup (gpsimd only) -------------
            warm_sb = consts.tile([1, 32], F32)
            nc.vector.memset(warm_sb[:, :], 0.0)
            wi = dram.tile([1, 32], F32, tag="wi")
            wo = dram.tile([1, 32], F32, tag="wo")
            nc.scalar.dma_start(out=wi[:, :], in_=warm_sb[:, :])
            nc.gpsimd.collective_compute(
                "AllReduce", mybir.AluOpType.add,
                replica_groups=GROUPS,
                ins=[wi[:, :].opt()],
                outs=[wo[:, :].opt()],
            )

            # ---------------- weight conversions ---------------------------
            wq = consts.tile([128, 2, 384], F32R, tag="wq")
            nc.vector.tensor_copy(wq[:, :, :], wq_f[:, :, :])
            kmem = consts.tile([128, MEM], BF16, tag="kmem")
            nc.vector.tensor_copy(kmem[:, :], memk_f[:, :])
            nc.vector.tensor_copy(vstag[NJT][0:MEM, :, :], memv_f[:, :, :])
            woutA = consts.tile([128, DIM], BF16, tag="woutA")
            woutB = consts.tile([128, DIM], BF16, tag="woutB")
            nc.vector.tensor_copy(woutA[64:128, :], woutA_f[64:128, :])
            nc.vector.tensor_copy(woutB[64:128, :], woutB_f[64:128, :])
            wouts = [woutA, woutB]

            # ---------------- prep: RMS stats + qkv ------------------------
            qb = [None] * NCH
            kb = [None] * NCH
            sigc_b = consts.tile([128, 2 * NJT], BF16, tag="sigc_b")

            for ci, (c0, cw) in enumerate(CHUNKS):
                njs = cw // 128
                xsq = wk.tile([128, 2, 512], F32R, tag="xsq", name=f"xsq_{ci}")
                for kt in range(2):
                    nc.scalar.activation(xsq[:, kt, 0:cw],
                                         xs[kt][ci][:, 0:cw], SQUARE)
                ssq = ps_a.tile([128, 512], F32, tag="a", name=f"ssq_{ci}")
                for kt in range(2):
                    nc.tensor.matmul(ssq[:, 0:cw], ones_r[:, :],
                                     xsq[:, kt, 0:cw],
                                     start=(kt == 0), stop=(kt == 1))
                # rows: sigma = 16/l2 (sqrt then fast reciprocal)
                sinvr = big.tile([128, 512], F32, tag=f"sr{ci}", name=f"sr{ci}")
                nc.scalar.activation(sinvr[:, 0:cw], ssq[:, 0:cw], SQRT,
                                     scale=1.0 / 256.0)
                nc.vector.reciprocal_approx_fast(sinvr[:, 0:cw],
                                                 sinvr[:, 0:cw])
                # column-form sum of squares (per-position, on partitions).
                # fp32r moving FD must be even -> duplicated 2-wide columns.
                sgp = ps_a.tile([128, 512], F32, tag="a", name=f"sgp_{ci}")
                for js in range(njs):
                    for kt in range(2):
                        nc.tensor.matmul(
                            sgp[:, 2 * js:2 * js + 2],
                            xsq[:, kt, js * 128:js * 128 + 128],
                            ones_r[:, 0:2],
                            start=(kt == 0), stop=(kt == 1),
                        )
                # 1/sigma = l2/16 = sqrt(ssq/256), keys on partitions
                nc.scalar.activation(siginv[:, ci * 8:ci * 8 + 2 * njs],
                                     sgp[:, 0:2 * njs], SQRT,
                                     scale=1.0 / 256.0)
                # f32r copy of x for the qkv matmuls (consumer needs rounding)
                xrt = wk.tile([128, 2, 512], F32R, tag="xr", name=f"xr_{ci}")
                for kt in range(2):
                    nc.vector.tensor_copy(xrt[:, kt, 0:cw],
                                          xs[kt][ci][:, 0:cw])
                xr = [xrt[:, 0, :], xrt[:, 1, :]]
                # q, k projections (normalized at readout); v direct-transposed
                qk = ps_s.tile([128, 2, 512], F32, tag="s", name=f"qk_{ci}")
                for m in range(2):  # 0 = q, 1 = k
                    for kt in range(2):
                        nc.tensor.matmul(
                            qk[:, m, 0:cw],
                            wq[:, kt, m * 128:m * 128 + 128],
                            xr[kt][:, 0:cw],
                            start=(kt == 0), stop=(kt == 1),
                        )
                vps = ps_a.tile([128, 4, 2, 64], F32, tag="a", name=f"vps_{ci}")
                for js in range(njs):
                    for kt in range(2):
                        nc.tensor.matmul(
                            vps[:, js, :, :],
                            xr[kt][:, js * 128:js * 128 + 128],
                            wq[:, kt, 256:384],
                            start=(kt == 0), stop=(kt == 1),
                        )
                qb[ci] = big.tile([128, 512], BF16, tag=f"q{ci}", name=f"qb{ci}")
                kb[ci] = big.tile([128, 512], BF16, tag=f"k{ci}", name=f"kb{ci}")
                nc.vector.tensor_mul(qb[ci][:, 0:cw], qk[:, 0, 0:cw],
                                     sinvr[:, 0:cw])
                nc.vector.tensor_mul(kb[ci][:, 0:cw], qk[:, 1, 0:cw],
                                     sinvr[:, 0:cw])
                for js in range(njs):
                    jt = ci * 4 + js
                    nc.vector.tensor_copy(
                        vstag[jt][:, :, 64:128], vps[:, js, :, :])

            # ---------------- sigma columns (single Ln batch) --------------
            # siginv holds 1/sigma per key (duplicated cols). Staging col 0
            # needs 1/sigma (bf16); ACT exp bias needs ln(sigma) =
            # -ln(1/sigma); DVE exp needs B + A*ln(sigma).
            nc.vector.tensor_copy(sigc_b[:, :], siginv[:, :])
            nc.scalar.activation(lnss[:, :], siginv[:, :], LN)
            nc.vector.tensor_scalar(out=lnsig[:, 0:2 * NJT], in0=lnss[:, :],
                                    scalar1=-1.0, scalar2=None, op0=MULT)
            nc.vector.tensor_scalar(out=dve_bias[:, :], in0=lnss[:, :],
                                    scalar1=-A_SCH, scalar2=B_SCH,
                                    op0=MULT, op1=ADD)
            for jt in range(NJT):
                nc.vector.tensor_copy(
                    vstag[jt][:, :, 0:1],
                    sigc_b[:, 2 * jt:2 * jt + 1].to_broadcast((128, 2, 1)))

            # ---------------- attention ------------------------------------
            bis = [dram.tile([2, 128, cw], BF16, tag=f"bi{ci}", name=f"bi{ci}")
                   for ci, (c0, cw) in enumerate(CHUNKS)]
            bos = [dram.tile([DHEAD, cw], BF16, tag=f"bo{ci}", name=f"bo{ci}")
                   for ci, (c0, cw) in enumerate(CHUNKS)]
            accs_by_ci = [None] * NCH
            fin = {}

            def emit_sim(ci, jt):
                c0, cw = CHUNKS[ci]
                s_ps = ps_s.tile([128, 2, 512], F32, tag="s",
                                 name=f"s_{ci}_{jt}")
                if jt < NJT:
                    km = 128
                    kc, off = _jt_slice(jt)
                    klhs = [kb[kc][64 * h:64 * h + 64, off:off + 128]
                            for h in range(2)]
                else:
                    km = MEM
                    klhs = [kmem[64 * h:64 * h + 64, :] for h in range(2)]
                for h in range(2):
                    nc.tensor.matmul(
                        s_ps[0:km, h, 0:cw],
                        klhs[h],
                        qb[ci][64 * h:64 * h + 64, 0:cw],
                        start=True, stop=True,
                    )
                return s_ps, km

            def emit_exp(ci, jt, s_ps, km):
                c0, cw = CHUNKS[ci]
                if jt in DVE_JTS:
                    P = wk.tile([128, 2, 512], U16, tag="Pd",
                                name=f"Pd_{ci}_{jt}")
                    nc.vector.tensor_scalar(
                        out=P[:, :, 0:cw], in0=s_ps[:, :, 0:cw],
                        scalar1=A_SCH, scalar2=dve_bias[:, 2 * jt:2 * jt + 1],
                        op0=MULT, op1=ADD)
                    return P, True
                P = wk.tile([128, 2, 512], BF16, tag="Pa", name=f"Pa_{ci}_{jt}")
                nc.scalar.activation(P[0:km, :, 0:cw], s_ps[0:km, :, 0:cw],
                                     EXP, bias=lnsig[0:km, 2 * jt:2 * jt + 1])
                return P, False

            def emit_attnv(ci, jt, P, km, cast):
                c0, cw = CHUNKS[ci]
                accs = accs_by_ci[ci]
                for h in range(2):
                    rhs = P[0:km, h, 0:cw]
                    if cast:
                        rhs = rhs.bitcast(BF16)
                    nc.tensor.matmul(
                        accs[h][:, 0:cw],
                        vstag[jt][0:km, h, :],
                        rhs,
                        start=(jt == 0), stop=(jt == NJT),
                        skip_group_check=True,
                    )

            def emit_fin_a(ci):
                """recip + partition broadcasts for chunk ci (DVE+gpsimd)."""
                c0, cw = CHUNKS[ci]
                accs = accs_by_ci[ci]
                rec = wk.tile([128, 2, 512], F32, tag="rec", name=f"rec_{ci}")
                rb = wk.tile([128, 2, 512], F32, tag="rb", name=f"rb_{ci}")
                for h in range(2):
                    nc.vector.reciprocal_approx_fast(
                        rec[0:1, h, 0:cw], accs[h][0:1, 0:cw])
                    nc.gpsimd.partition_broadcast(rb[:, h, 0:cw],
                                                  rec[0:1, h, 0:cw])
                fin[ci] = (rec, rb)

            def emit_fin_b(ci):
                """normalize oT (DVE)."""
                c0, cw = CHUNKS[ci]
                accs = accs_by_ci[ci]
                rec, rb = fin[ci]
                oT = wk.tile([128, 2, 512], BF16, tag="oT", name=f"oT_{ci}")
                for h in range(2):
                    nc.vector.tensor_mul(oT[64:128, h, 0:cw],
                                         accs[h][64:128, 0:cw],
                                         rb[64:128, h, 0:cw])
                fin[ci] = oT

            def emit_fin_c(ci):
                """out-projection (PE)."""
                c0, cw = CHUNKS[ci]
                oT = fin[ci]
                op = ps_s.tile([128, 2, 512], F32, tag="s", name=f"op_{ci}")
                for mt in range(2):
                    for h in range(2):
                        nc.tensor.matmul(
                            op[:, mt, 0:cw],
                            wouts[h][64:128, mt * 128:mt * 128 + 128],
                            oT[64:128, h, 0:cw],
                            start=(h == 0), stop=(h == 1),
                        )
                fin[ci] = op

            def emit_fin_d(ci):
                """osb copy (DVE) + DMA to DRAM (sync) + RS (gpsimd)."""
                c0, cw = CHUNKS[ci]
                op = fin[ci]
                osb = wk.tile([128, 2, 512], BF16, tag="osb", name=f"osb_{ci}")
                nc.vector.tensor_copy(osb[:, :, 0:cw], op[:, :, 0:cw])
                for mt in range(2):
                    nc.sync.dma_start(out=bis[ci][mt, :, :],
                                      in_=osb[:, mt, 0:cw])
                nc.gpsimd.collective_compute(
                    "ReduceScatter", mybir.AluOpType.add,
                    replica_groups=GROUPS,
                    ins=[bis[ci][:, :, :].opt()],
                    outs=[bos[ci][:, :].opt()],
                )

            for ci, (c0, cw) in enumerate(CHUNKS):
                acc0 = ps_a.tile([128, 512], F32, tag="a", name=f"acc0_{ci}")
                acc1 = ps_a.tile([128, 512], F32, tag="a", name=f"acc1_{ci}")
                accs_by_ci[ci] = [acc0, acc1]
                pend = None
                for jt in range(NJT + 1):
                    s_ps, km = emit_sim(ci, jt)
                    if pend is not None:
                        emit_attnv(ci, *pend)
                    # finish hooks for the previous chunk BEFORE this jt's
                    # exp: a hook's DVE work must not queue behind an exp
                    # that transitively waits on a psum buffer the hook
                    # itself frees (deadlock otherwise).
                    if ci > 0:
                        if jt == 3:
                            emit_fin_a(ci - 1)
                        elif jt == 5:
                            emit_fin_b(ci - 1)
                        elif jt == 8:
                            emit_fin_c(ci - 1)
                        elif jt == 10:
                            emit_fin_d(ci - 1)
                    P, cast = emit_exp(ci, jt, s_ps, km)
                    pend = (jt, P, km, cast)
                emit_attnv(ci, *pend)
            emit_fin_a(NCH - 1)
            emit_fin_b(NCH - 1)
            emit_fin_c(NCH - 1)
            emit_fin_d(NCH - 1)
            for ci, (c0, cw) in enumerate(CHUNKS):
                nc.sync.dma_start(out=out_d[:, c0:c0 + cw], in_=bos[ci][:, :])
    nc.compile()
    return nc


_NC = None
_last_in_maps = None


def _get_nc():
    global _NC
    if _NC is None:
        _NC = build()
    return _NC


def make_in_maps(x, gamma, mem_kv, w_qkv, w_out):
    x = np.asarray(x, np.float32)
    gamma = np.asarray(gamma, np.float32).reshape(DIM)
    mem_kv = np.asarray(mem_kv, np.float32)
    w_qkv = np.asarray(w_qkv, np.float32)
    w_out = np.asarray(w_out, np.float32)

    g1 = 1.0 + gamma  # [256]
    scale = DHEAD ** -0.5
    in_maps = []
    for core in range(8):
        b, g = core // 4, core % 4
        hA, hB = 2 * g, 2 * g + 1
        blocks = []
        for t in range(3):  # q, k, v
            for h in (hA, hB):
                wblk = w_qkv[t * HID + h * DHEAD: t * HID + (h + 1) * DHEAD, :]
                if t == 0:
                    wblk = wblk * scale
                blocks.append(wblk.T)  # [256, 64]
        wqkvT = np.concatenate(blocks, axis=1) * g1[:, None]  # [256, 384]
        memk = np.concatenate(
            [mem_kv[0, hA].T, mem_kv[0, hB].T], axis=0)  # [128, 4]
        # mem staging: [4, 2, 128] = [1/sigma(=1) | zeros(63) | v(64)]
        memvst = np.zeros((MEM, 2, 128), np.float32)
        memvst[:, :, 0] = 1.0
        memvst[:, 0, 64:128] = mem_kv[1, hA]
        memvst[:, 1, 64:128] = mem_kv[1, hB]
        woutT = np.stack(
            [w_out[:, hA * DHEAD:(hA + 1) * DHEAD].T,
             w_out[:, hB * DHEAD:(hB + 1) * DHEAD].T], axis=0)  # [2, 64, 256]
        in_maps.append({
            "x": np.ascontiguousarray(x[b].reshape(DIM, N)),
            "wqkv": np.ascontiguousarray(wqkvT),
            "memk": np.ascontiguousarray(memk),
            "memvst": np.ascontiguousarray(memvst),
            "woutT": np.ascontiguousarray(woutT),
        })
    return in_maps


def kernel(x, gamma, mem_kv, w_qkv, w_out):
    global _last_in_maps
    in_maps = make_in_maps(x, gamma, mem_kv, w_qkv, w_out)
    _last_in_maps = in_maps
    nc = _get_nc()
    res = run_bass_kernel_spmd(nc, in_maps, core_ids=list(range(8)))
    out = np.empty((2, DIM, N), np.float32)
    for core in range(8):
        b, g = core // 4, core % 4
        out[b, 64 * g:64 * g + 64, :] = np.asarray(
            res.results[core]["out"], dtype=np.float32)
    return out.reshape(2, DIM, 48, 48)
